# revision 9
# baseline (speedup 1.0000x reference)
"""Trainium2 Bass kernel for the 6-layer bigram transformer + CE loss.

Sharding: DP=4 over batch x TP=2 (heads / FFN hidden / vocab), 8 cores.
Matmul operands are float32r (full-rate fp32 matmul on TRN2) except the
attention-score and FFN-second-matmul operands which are bf16 for SBUF
headroom.  Activations/stats are fp32.  Activation layout is tokens-major
[t(partitions), e(free)] for LN/residual work; matmul operands are built
feature-major via PE transposes of the LN outputs.  Attention computes
scores^T per head ([s(part), t(free)]) so causal masking restricts the
free range; softmax denominators come from a ones-vector matmul
(two-pass: probabilities are recomputed after the denominator pass) and
are applied after a DRAM-bounce partition broadcast.  CE uses
exp-without-max (|logit| < 4) with fused ACT accumulation; target logits
are gathered back from the written logits via indirect DMA.
"""
import os
import sys

sys.path.insert(0, "/opt/trn_rl_repo")

import numpy as np

B, T, E, H, DH, L, V, FF = 4, 1024, 1024, 16, 64, 6, 32000, 4096
P = 128
NT = T // P            # 8 t-tiles
NE = E // P            # 8 e-chunks
TPD = 2                # tensor parallel degree
HL = H // TPD          # 8 local heads
FFL = FF // TPD        # 2048 local ff
VL = V // TPD          # 16000 local vocab
VCW = 320              # vocab chunk width (>=256 keeps f32r full rate)
VC = VL // VCW         # 50 chunks
SCALE = DH ** -0.5
N_CORES = 8
RG_TP = [[0, 1], [2, 3], [4, 5], [6, 7]]
KLAYERS = int(os.environ.get("KLAYERS", str(L)))


# ---------------------------------------------------------------- host prep
def _prep_core(inputs, core):
    b, r = core // TPD, core % TPD
    f32 = np.float32
    import ml_dtypes
    bf16 = ml_dtypes.bfloat16
    x = np.asarray(inputs["x"]).astype(np.int32)[b]
    tgt = np.asarray(inputs["targets"]).astype(np.int64)[b]
    vofs = r * VL
    rel = tgt - vofs
    ok = (rel >= 0) & (rel < VL)
    gidx = np.where(ok, np.arange(T, dtype=np.int64) * VL + rel, 1 << 30)

    hsl = slice(r * HL, (r + 1) * HL)
    fsl = slice(r * FFL, (r + 1) * FFL)
    vsl = slice(vofs, vofs + VL)

    wq = np.asarray(inputs["Wq"])[:, hsl]
    wk = np.asarray(inputs["Wk"])[:, hsl]
    wv = np.asarray(inputs["Wv"])[:, hsl]
    wq_t = wq.reshape(L, HL * DH, E).transpose(0, 2, 1)
    wk_t = wk.reshape(L, HL * DH, E).transpose(0, 2, 1)
    wv_t = wv.reshape(L, HL * DH, E).transpose(0, 2, 1)
    wo = np.asarray(inputs["Wo"])
    wo_t = wo[:, :, r * HL * DH:(r + 1) * HL * DH].transpose(0, 2, 1)
    w1 = np.asarray(inputs["W1"])[:, fsl]
    w1_t = w1.transpose(0, 2, 1)
    w2 = np.asarray(inputs["W2"])
    w2_t = w2[:, :, fsl].transpose(0, 2, 1)
    wh = np.asarray(inputs["Wh"])[vsl]
    wh_t = wh.T

    c = np.ascontiguousarray
    return dict(
        x_idx=c(x.reshape(NT, P, 1).astype(np.int32)),
        tgt_gidx=c(gidx.reshape(NT, P, 1).astype(np.int32)),
        tok_emb=c(np.asarray(inputs["tok_emb"], f32)),
        pos_emb=c(np.asarray(inputs["pos_emb"], f32)),
        wq_t=c(wq_t, dtype=f32), wk_t=c(wk_t, dtype=f32),
        wv_t=c(wv_t, dtype=f32), wo_t=c(wo_t, dtype=f32),
        w1_t=c(w1_t, dtype=f32),
        w2_t=c(w2_t.astype(bf16)),
        wh_t=c(wh_t, dtype=f32),
        ln1_g=c(np.asarray(inputs["ln1_g"], f32).reshape(L, 1, E)),
        ln1_b=c(np.asarray(inputs["ln1_b"], f32).reshape(L, 1, E)),
        ln2_g=c(np.asarray(inputs["ln2_g"], f32).reshape(L, 1, E)),
        ln2_b=c(np.asarray(inputs["ln2_b"], f32).reshape(L, 1, E)),
        bo_r=c(np.asarray(inputs["bo"], f32).reshape(L, 1, E)),
        b2_r=c(np.asarray(inputs["b2"], f32).reshape(L, 1, E)),
        b1_r=c(np.asarray(inputs["b1"], f32)[:, fsl].reshape(L, FFL // P, P, 1)),
        lnf_g=c(np.asarray(inputs["lnf_g"], f32).reshape(1, E)),
        lnf_b=c(np.asarray(inputs["lnf_b"], f32).reshape(1, E)),
        bh_r=c(np.asarray(inputs["bh"], f32)[vsl].reshape(1, VL)),
        ones128=np.ones((P, 1), f32),
        ident=np.eye(P, dtype=f32),
    )


# ---------------------------------------------------------------- device build
def _build_nc():
    import concourse.bacc as bacc
    import concourse.bass as bass
    import concourse.bass_isa as bass_isa
    import concourse.tile as tile
    from concourse import mybir
    from concourse.tile import add_dep_helper

    f32 = mybir.dt.float32
    f32r = mybir.dt.float32r
    bf16 = mybir.dt.bfloat16
    i32 = mybir.dt.int32
    AF = mybir.ActivationFunctionType
    ALU = mybir.AluOpType
    AX = mybir.AxisListType

    nc = bacc.Bacc("TRN2", target_bir_lowering=False, debug=False,
                   num_devices=N_CORES)

    def din(name, shape, dt):
        return nc.dram_tensor(name, shape, dt, kind="ExternalInput").ap()

    x_idx = din("x_idx", [NT, P, 1], i32)
    tgt_gidx = din("tgt_gidx", [NT, P, 1], i32)
    tok_emb = din("tok_emb", [V, E], f32)
    pos_emb = din("pos_emb", [T, E], f32)
    wq_t = din("wq_t", [L, E, HL * DH], f32r)
    wk_t = din("wk_t", [L, E, HL * DH], f32r)
    wv_t = din("wv_t", [L, E, HL * DH], f32r)
    wo_t = din("wo_t", [L, HL * DH, E], f32r)
    w1_t = din("w1_t", [L, E, FFL], f32r)
    w2_t = din("w2_t", [L, FFL, E], bf16)
    wh_t = din("wh_t", [E, VL], f32r)
    ln1_g = din("ln1_g", [L, 1, E], f32)
    ln1_b = din("ln1_b", [L, 1, E], f32)
    ln2_g = din("ln2_g", [L, 1, E], f32)
    ln2_b = din("ln2_b", [L, 1, E], f32)
    bo_r = din("bo_r", [L, 1, E], f32)
    b2_r = din("b2_r", [L, 1, E], f32)
    b1_r = din("b1_r", [L, FFL // P, P, 1], f32)
    lnf_g = din("lnf_g", [1, E], f32)
    lnf_b = din("lnf_b", [1, E], f32)
    bh_r = din("bh_r", [1, VL], f32)
    ones128 = din("ones128", [P, 1], f32r)
    ident = din("ident", [P, P], f32)

    logits_loc = nc.dram_tensor("logits_loc", [T, VL], f32,
                                kind="ExternalOutput").ap()
    loss_part = nc.dram_tensor("loss_part", [1, 1], f32,
                               kind="ExternalOutput").ap()

    from contextlib import ExitStack
    with tile.TileContext(nc) as tc, ExitStack() as stk:
        pool = lambda **kw: stk.enter_context(tc.tile_pool(**kw))
        cn = pool(name="cn", bufs=1)
        hp = pool(name="hp", bufs=1)
        atp = pool(name="atp", bufs=1)
        qkv = pool(name="qkv", bufs=1)
        attp = pool(name="attp", bufs=2)
        ftp = pool(name="ftp", bufs=1)
        work = pool(name="work", bufs=3)
        small = pool(name="small", bufs=4)
        gbp = pool(name="gbp", bufs=2)
        dsbp = pool(name="dsbp", bufs=4)
        wsp = pool(name="wsp", bufs=4)
        whp = pool(name="whp", bufs=3)
        lgp = pool(name="lgp", bufs=2)
        ps = pool(name="ps", bufs=4, space="PSUM")
        pso = pool(name="pso", bufs=2, space="PSUM")
        psse = pool(name="psse", bufs=2, space="PSUM")
        stp = pool(name="stp", bufs=1)
        dram = pool(name="dram", bufs=8, space="DRAM")

        # ---------------- constants
        identt = cn.tile([P, P], f32)
        nc.sync.dma_start(out=identt[:], in_=ident[:])
        onest = cn.tile([P, 1], f32r)
        nc.sync.dma_start(out=onest[:], in_=ones128[:])
        epst = cn.tile([P, 1], f32)
        nc.vector.memset(epst[:], 1e-5)

        # h resident tokens-major [P, NT, E]
        h = hp.tile([P, NT, E], f32)
        # feature-major transposed activations [P, NE, T] (f32r)
        aT = atp.tile([P, NE, T], f32r, tag="aT")

        # ---------------- embedding
        for tt in range(NT):
            ixt = small.tile([P, 1], i32, tag="idx")
            nc.sync.dma_start(out=ixt[:], in_=x_idx[tt])
            nc.gpsimd.indirect_dma_start(
                out=h[:, tt, :], out_offset=None, in_=tok_emb[:],
                in_offset=bass.IndirectOffsetOnAxis(ap=ixt[:, :1], axis=0))
            pt = work.tile([P, E], f32, tag="work")
            nc.sync.dma_start(out=pt[:], in_=pos_emb[tt * P:(tt + 1) * P, :])
            nc.vector.tensor_add(out=h[:, tt, :], in0=h[:, tt, :], in1=pt[:])

        # ---------------- helpers
        def layernorm_transpose(gap, bap):
            """LN of h (tokens-major) -> aT (feature-major f32r)."""
            gt = gbp.tile([P, E], f32, tag="gb")
            nc.sync.dma_start(out=gt[:], in_=gap.to_broadcast((P, E)))
            bt = gbp.tile([P, E], f32, tag="gb")
            nc.sync.dma_start(out=bt[:], in_=bap.to_broadcast((P, E)))
            for tt in range(NT):
                st = small.tile([P, 2, 6], f32, tag="bn")
                nc.vector.bn_stats(out=st[:, 0, :], in_=h[:, tt, 0:512])
                nc.vector.bn_stats(out=st[:, 1, :], in_=h[:, tt, 512:1024])
                mv = small.tile([P, 2], f32, tag="mv")
                nc.vector.bn_aggr(out=mv[:], in_=st[:])
                r = small.tile([P, 1], f32, tag="r")
                nc.scalar.activation(out=r[:], in_=mv[:, 1:2], func=AF.Sqrt,
                                     bias=epst[:], scale=1.0)
                nc.vector.reciprocal(out=r[:], in_=r[:])
                nm = small.tile([P, 1], f32, tag="nm")
                nc.vector.tensor_mul(out=nm[:], in0=mv[:, 0:1], in1=r[:])
                nc.vector.tensor_scalar_mul(out=nm[:], in0=nm[:], scalar1=-1.0)
                a = work.tile([P, E], f32, tag="work")
                nc.scalar.activation(out=a[:], in_=h[:, tt, :], func=AF.Identity,
                                     scale=r[:], bias=nm[:])
                nc.vector.tensor_mul(out=a[:], in0=a[:], in1=gt[:])
                nc.vector.tensor_add(out=a[:], in0=a[:], in1=bt[:])
                for eb in range(NE):
                    pt_ = ps.tile([P, P], f32, tag="mm")
                    nc.tensor.transpose(out=pt_[:], in_=a[:, eb * P:(eb + 1) * P],
                                        identity=identt[:])
                    nc.scalar.copy(out=aT[:, eb, tt * P:(tt + 1) * P], in_=pt_[:])

        def allreduce_tile(src_sbuf_ap, dst_sbuf_ap):
            """Pair all-reduce of one [P, E] tile via DRAM bounce."""
            bi = dram.tile([P, E], f32, tag="arin")
            bo = dram.tile([P, E], f32, tag="arout")
            nc.sync.dma_start(out=bi[:], in_=src_sbuf_ap)
            nc.gpsimd.collective_compute(
                "AllReduce", ALU.add, replica_groups=RG_TP,
                ins=[bi.opt()], outs=[bo.opt()])
            nc.sync.dma_start(out=dst_sbuf_ap, in_=bo[:])

        # last attention s-tile contributing to each 512-wide t-chunk
        LAST_SI = {tc: max(si for si in range(NT) if si * P < (tc + 1) * 512)
                   for tc in range(2)}

        # ---------------- transformer layers
        for l in range(KLAYERS):
            layernorm_transpose(ln1_g[l], ln1_b[l])

            # ---- QKV ----
            qT = qkv.tile([P, 4, T], bf16, tag="qT")
            kT = qkv.tile([P, 4, T], bf16, tag="kT")
            vt = qkv.tile([P, NT, HL * DH], f32r, tag="v")
            for n in range(2):
                for m in range(4):
                    pq = ps.tile([P, 512], f32, tag="mm")
                    pk = ps.tile([P, 512], f32, tag="mm")
                    for ke in range(NE):
                        wqt = wsp.tile([P, P], f32r, tag="ws")
                        nc.sync.dma_start(
                            out=wqt[:],
                            in_=wq_t[l][ke * P:(ke + 1) * P, m * P:(m + 1) * P])
                        wkt = wsp.tile([P, P], f32r, tag="ws")
                        nc.sync.dma_start(
                            out=wkt[:],
                            in_=wk_t[l][ke * P:(ke + 1) * P, m * P:(m + 1) * P])
                        rr = aT[:, ke, n * 512:(n + 1) * 512]
                        nc.tensor.matmul(out=pq[:], lhsT=wqt[:], rhs=rr,
                                         start=(ke == 0), stop=(ke == NE - 1))
                        nc.tensor.matmul(out=pk[:], lhsT=wkt[:], rhs=rr,
                                         start=(ke == 0), stop=(ke == NE - 1))
                    nc.scalar.copy(out=qT[:, m, n * 512:(n + 1) * 512], in_=pq[:])
                    nc.scalar.copy(out=kT[:, m, n * 512:(n + 1) * 512], in_=pk[:])
            for tt in range(NT):
                pv = ps.tile([P, 512], f32, tag="mm")
                for ke in range(NE):
                    wvt = wsp.tile([P, HL * DH], f32r, tag="ws")
                    nc.sync.dma_start(out=wvt[:],
                                      in_=wv_t[l][ke * P:(ke + 1) * P, :])
                    nc.tensor.matmul(out=pv[:], lhsT=aT[:, ke, tt * P:(tt + 1) * P],
                                     rhs=wvt[:], start=(ke == 0),
                                     stop=(ke == NE - 1))
                nc.scalar.copy(out=vt[:, tt, :], in_=pv[:])

            # ---- attention per local head (two passes over s-tiles) ----
            oT = qkv.tile([P, 4, T], f32r, tag="oT")
            for hh in range(HL):
                po, pt_i = (hh % 2) * 64, hh // 2
                kh = kT[po:po + 64, pt_i, :]
                qh = qT[po:po + 64, pt_i, :]

                def make_att(si):
                    """scores^T -> exp -> causal mask for s-tile si."""
                    t0 = si * P
                    at_ = attp.tile([P, T], f32r, tag="att")
                    for c0 in range(t0, T, 512):
                        w = min(512, T - c0)
                        psc = ps.tile([P, 512], f32, tag="mm")
                        nc.tensor.matmul(out=psc[:, :w],
                                         lhsT=kh[:, t0:t0 + P],
                                         rhs=qh[:, c0:c0 + w],
                                         start=True, stop=True)
                        nc.scalar.activation(out=at_[:, c0 - t0:c0 - t0 + w],
                                             in_=psc[:, :w], func=AF.Exp,
                                             scale=float(SCALE))
                    nc.gpsimd.affine_select(
                        out=at_[:, 0:P], in_=at_[:, 0:P],
                        compare_op=ALU.is_ge, fill=0.0, base=0,
                        pattern=[[1, P]], channel_multiplier=-1)
                    return at_

                # pass 1: denominators via ones-matmul
                pses = [psse.tile([1, 512], f32, tag="se", name=f"pse{hh}_{_i}") for _i in range(2)]
                for si in range(NT):
                    t0 = si * P
                    at_ = make_att(si)
                    for tc in range(2):
                        c0 = tc * 512
                        lo = max(t0, c0)
                        if lo >= c0 + 512:
                            continue
                        nc.tensor.matmul(out=pses[tc][:, lo - c0:512],
                                         lhsT=onest[:],
                                         rhs=at_[:, lo - t0:c0 + 512 - t0],
                                         start=(si == 0),
                                         stop=(si == LAST_SI[tc]))
                set_ = work.tile([1, T], f32, tag="work")
                for tc in range(2):
                    nc.scalar.copy(out=set_[0:1, tc * 512:(tc + 1) * 512],
                                   in_=pses[tc][:])
                nc.vector.reciprocal(out=set_[0:1, :], in_=set_[0:1, :])
                seb = dram.tile([1, T], f32, tag="seb")
                nc.sync.dma_start(out=seb[:], in_=set_[0:1, :])
                rse = work.tile([P, T], f32, tag="work")
                nc.sync.dma_start(out=rse[:],
                                  in_=seb[0:1, :].to_broadcast((P, T)))

                # pass 2: recompute probs, normalize, accumulate o^T
                pos_ = [pso.tile([64, 512], f32, tag="o", name=f"pos{hh}_{_i}") for _i in range(2)]
                for si in range(NT):
                    t0 = si * P
                    at_ = make_att(si)
                    nc.vector.tensor_mul(out=at_[:, :T - t0],
                                         in0=at_[:, :T - t0],
                                         in1=rse[:, t0:])
                    for tc in range(2):
                        c0 = tc * 512
                        lo = max(t0, c0)
                        if lo >= c0 + 512:
                            continue
                        nc.tensor.matmul(
                            out=pos_[tc][:, lo - c0:512],
                            lhsT=vt[:, si, hh * 64:(hh + 1) * 64],
                            rhs=at_[:, lo - t0:c0 + 512 - t0],
                            start=(si == 0), stop=(si == LAST_SI[tc]))
                for tc in range(2):
                    nc.scalar.copy(out=oT[po:po + 64, pt_i,
                                          tc * 512:(tc + 1) * 512],
                                   in_=pos_[tc][:])

            # ---- output projection (partial) + AR + residual ----
            bot = gbp.tile([P, E], f32, tag="gb")
            nc.sync.dma_start(out=bot[:], in_=bo_r[l].to_broadcast((P, E)))
            for tt in range(NT):
                dsb = dsbp.tile([P, E], f32, tag="dsb")
                for n in range(2):
                    pd = ps.tile([P, 512], f32, tag="mm")
                    for kd in range(4):
                        wot = wsp.tile([P, 512], f32r, tag="ws")
                        nc.sync.dma_start(
                            out=wot[:],
                            in_=wo_t[l][kd * P:(kd + 1) * P,
                                        n * 512:(n + 1) * 512])
                        nc.tensor.matmul(out=pd[:],
                                         lhsT=oT[:, kd, tt * P:(tt + 1) * P],
                                         rhs=wot[:], start=(kd == 0),
                                         stop=(kd == 3))
                    nc.scalar.copy(out=dsb[:, n * 512:(n + 1) * 512], in_=pd[:])
                rt = work.tile([P, E], f32, tag="work")
                allreduce_tile(dsb[:], rt[:])
                nc.vector.tensor_add(out=h[:, tt, :], in0=h[:, tt, :], in1=rt[:])
                nc.vector.tensor_add(out=h[:, tt, :], in0=h[:, tt, :], in1=bot[:])

            # ---- FFN ----
            layernorm_transpose(ln2_g[l], ln2_b[l])
            b2t = gbp.tile([P, E], f32, tag="gb")
            nc.sync.dma_start(out=b2t[:], in_=b2_r[l].to_broadcast((P, E)))
            for tc in range(2):
                c0 = tc * 512
                dsbs = [dsbp.tile([P, E], f32, tag="dsb", name=f"dsbs{l}_{tc}_{_i}") for _i in range(4)]
                for slab in range(2):
                    s0 = slab * 8
                    fT = ftp.tile([P, 8, 512], bf16, tag="fT")
                    for m in range(8):
                        pf = ps.tile([P, 512], f32, tag="mm")
                        for ke in range(NE):
                            w1t = wsp.tile([P, P], f32r, tag="ws")
                            nc.sync.dma_start(
                                out=w1t[:],
                                in_=w1_t[l][ke * P:(ke + 1) * P,
                                            (s0 + m) * P:(s0 + m + 1) * P])
                            nc.tensor.matmul(out=pf[:], lhsT=w1t[:],
                                             rhs=aT[:, ke, c0:c0 + 512],
                                             start=(ke == 0), stop=(ke == NE - 1))
                        b1t = small.tile([P, 1], f32, tag="b1")
                        nc.sync.dma_start(out=b1t[:], in_=b1_r[l, s0 + m])
                        nc.scalar.activation(out=fT[:, m, :], in_=pf[:],
                                             func=AF.Relu, bias=b1t[:])
                    for e in range(2):
                        pds = [ps.tile([P, 512], f32, tag="mm",
                                           name=f"pds{l}_{tc}_{slab}_{e}_{_i}")
                                   for _i in range(4)]
                        for kf in range(8):
                            w2t = wsp.tile([P, 512], bf16, tag="ws")
                            nc.sync.dma_start(
                                out=w2t[:],
                                in_=w2_t[l][(s0 + kf) * P:(s0 + kf + 1) * P,
                                            e * 512:(e + 1) * 512])
                            for tt in range(4):
                                nc.tensor.matmul(
                                    out=pds[tt][:],
                                    lhsT=fT[:, kf, tt * P:(tt + 1) * P],
                                    rhs=w2t[:], start=(kf == 0), stop=(kf == 7))
                        for tt in range(4):
                            if slab == 0:
                                nc.scalar.copy(
                                    out=dsbs[tt][:, e * 512:(e + 1) * 512],
                                    in_=pds[tt][:])
                            else:
                                nc.vector.tensor_add(
                                    out=dsbs[tt][:, e * 512:(e + 1) * 512],
                                    in0=dsbs[tt][:, e * 512:(e + 1) * 512],
                                    in1=pds[tt][:])
                for tt in range(4):
                    gtt = tc * 4 + tt
                    rt = work.tile([P, E], f32, tag="work")
                    allreduce_tile(dsbs[tt][:], rt[:])
                    nc.vector.tensor_add(out=h[:, gtt, :], in0=h[:, gtt, :],
                                         in1=rt[:])
                    nc.vector.tensor_add(out=h[:, gtt, :], in0=h[:, gtt, :],
                                         in1=b2t[:])

        # ---------------- final LN + LM head + CE ----------------
        layernorm_transpose(lnf_g, lnf_b)  # hf^T into aT

        sestrip = stp.tile([P, NT, VC], f32, tag="strip")
        logit_stores = [[] for _ in range(NT)]
        for vc in range(VC):
            v0 = vc * VCW
            bht = gbp.tile([P, VCW], f32, tag="bh")
            nc.sync.dma_start(out=bht[:],
                              in_=bh_r[0:1, v0:v0 + VCW].to_broadcast((P, VCW)))
            wh_a = whp.tile([P, NE // 2, VCW], f32r, tag="wh")
            nc.sync.dma_start(out=wh_a[:],
                              in_=wh_t[0:E // 2, v0:v0 + VCW].rearrange(
                                  "(ne p) v -> p ne v", p=P))
            wh_b = whp.tile([P, NE // 2, VCW], f32r, tag="wh")
            nc.sync.dma_start(out=wh_b[:],
                              in_=wh_t[E // 2:E, v0:v0 + VCW].rearrange(
                                  "(ne p) v -> p ne v", p=P))
            for tt in range(NT):
                pl = ps.tile([P, VCW], f32, tag="mm")
                for ke in range(NE):
                    whh = wh_a if ke < 4 else wh_b
                    nc.tensor.matmul(out=pl[:],
                                     lhsT=aT[:, ke, tt * P:(tt + 1) * P],
                                     rhs=whh[:, ke % 4, :], start=(ke == 0),
                                     stop=(ke == NE - 1))
                lg = lgp.tile([P, VCW], f32, tag="lg")
                nc.vector.tensor_add(out=lg[:], in0=pl[:], in1=bht[:])
                nc.scalar.activation(out=pl[:], in_=lg[:], func=AF.Exp,
                                     accum_out=sestrip[:, tt, vc:vc + 1])
                st_i = nc.sync.dma_start(
                    out=logits_loc[tt * P:(tt + 1) * P, v0:v0 + VCW], in_=lg[:])
                logit_stores[tt].append(st_i)

        # sum strips -> per-token sumexp; pair-AR; lse
        se8 = small.tile([P, NT], f32, tag="se8")
        for tt in range(NT):
            nc.vector.reduce_sum(out=se8[:, tt:tt + 1], in_=sestrip[:, tt, :],
                                 axis=AX.X)
        sei = dram.tile([P, NT], f32, tag="sei")
        seo = dram.tile([P, NT], f32, tag="seo")
        nc.sync.dma_start(out=sei[:], in_=se8[:])
        nc.gpsimd.collective_compute("AllReduce", ALU.add, replica_groups=RG_TP,
                                     ins=[sei.opt()], outs=[seo.opt()])
        lse = small.tile([P, NT], f32, tag="lse")
        nc.sync.dma_start(out=lse[:], in_=seo[:])
        nc.scalar.activation(out=lse[:], in_=lse[:], func=AF.Ln)

        # gather target logits from written logits (flat view)
        tl = small.tile([P, NT], f32, tag="tl")
        nc.vector.memset(tl[:], 0.0)
        flat_logits = logits_loc.rearrange("t (v one) -> (t v) one", one=1)
        for tt in range(NT):
            gxt = small.tile([P, 1], i32, tag="idx")
            nc.sync.dma_start(out=gxt[:], in_=tgt_gidx[tt])
            g_i = nc.gpsimd.indirect_dma_start(
                out=tl[:, tt:tt + 1], out_offset=None, in_=flat_logits,
                in_offset=bass.IndirectOffsetOnAxis(ap=gxt[:, :1], axis=0),
                bounds_check=T * VL - 1, oob_is_err=False)
            for st_i in logit_stores[tt]:
                add_dep_helper(g_i.ins, st_i.ins,
                               reason="tl gather after logits stores")

        # partial loss: sum over tokens of 0.5*lse - tl
        pl_ = small.tile([P, NT], f32, tag="pl")
        nc.vector.tensor_scalar_mul(out=pl_[:], in0=lse[:], scalar1=0.5)
        nc.vector.tensor_sub(out=pl_[:], in0=pl_[:], in1=tl[:])
        pr = small.tile([P, 1], f32, tag="pr")
        nc.vector.reduce_sum(out=pr[:], in_=pl_[:], axis=AX.X)
        prr = small.tile([P, 1], f32, tag="prr")
        nc.gpsimd.partition_all_reduce(prr[:], pr[:], channels=P,
                                       reduce_op=bass_isa.ReduceOp.add)
        nc.sync.dma_start(out=loss_part[:], in_=prr[0:1, :])

    nc.compile()
    return nc


_NC_CACHE = None


def _get_nc():
    global _NC_CACHE
    if _NC_CACHE is None:
        _NC_CACHE = _build_nc()
    return _NC_CACHE


def kernel(**inputs):
    from concourse.bass_utils import run_bass_kernel_spmd

    nc = _get_nc()
    in_maps = [_prep_core(inputs, c) for c in range(N_CORES)]
    res = run_bass_kernel_spmd(nc, in_maps, list(range(N_CORES))).results

    logits = np.zeros((B, T, V), np.float32)
    loss_sum = 0.0
    for c in range(N_CORES):
        b, r = c // TPD, c % TPD
        logits[b, :, r * VL:(r + 1) * VL] = res[c]["logits_loc"]
        loss_sum += float(res[c]["loss_part"][0, 0])
    loss = np.float32(loss_sum / (B * T))
    return logits.reshape(B * T, V), loss
